# revision 5
# baseline (speedup 1.0000x reference)
"""DynamicCenterLoss on Trainium2 (Bass/Tile), 8-core SPMD.

Strategy: `batch` is sorted, so shard at batch boundaries -> core b owns
batch b (sizes ~N/8 +- <1%). Per core, every needed statistic is a
13-class one-hot segment reduction computed on the tensor engine:

    OUT[13, 65] = sum_n onehot(tgt_n)^T (x) [feat_n | 1]
      -> fsum[13,64] (per-class feature sums), ccnt[13] (per-class counts)

plus S = sum_n ||feat_n||^2, split between the Scalar engine
(activation Square + accum) and the Vector engine (tensor_tensor_reduce)
so that no single engine exceeds the DMA floor.  feat is cast to bf16
on the host, halving HBM traffic (the PE consumed bf16 anyway); DMA
lands directly in the interleaved [feat | 1] SBUF layout.  The intra
term uses  sum_n ||f_n - c_{t_n}||^2 = S - 2*sum_c c_c.fsum_c + sum_c
ccnt_c*||c_c||^2, so no per-point gather of centers is ever needed.
Padded rows (target=13) produce an all-zero one-hot row and zero
features, so they contribute nothing. The pairwise-center hinge loss is
computed per core on its own (13,64) stats; the host only averages the
8 per-batch scalars.
"""

import numpy as np
import ml_dtypes

import concourse.bass as bass
import concourse.bacc as bacc
import concourse.tile as tile
from concourse import mybir
from concourse.bass_utils import run_bass_kernel_spmd

P = 128
D = 64
C = 13
B = 8
N_CORES = 8
MARGIN = 0.5
INTRA_W = 1.0
INTER_W = 1.0
LOSS_W = 0.01
IGNORE = -1
TT = 64  # matmul steps (128-point chunks) per SBUF tile
NBUF = 6  # explicit ext buffers (ones column memset once, never rewritten)

f32 = mybir.dt.float32
bf16 = mybir.dt.bfloat16
i32 = mybir.dt.int32


def build_nc(T: int) -> bass.Bass:
    """Build the per-core Bass program. T = points per SBUF partition."""
    Npad = P * T
    # tile splits: small first tiles so the PE pipeline fills early
    splits = []
    _t0 = 0
    szs = [16, 48]
    while sum(szs) + TT <= T:
        szs.append(TT)
    for sz in szs:
        if _t0 >= T:
            break
        sz = min(sz, T - _t0)
        splits.append((_t0, sz))
        _t0 += sz
    if _t0 < T:
        splits.append((_t0, T - _t0))
    ntiles = len(splits)
    # square-pass split: first ~62% of chunks on ScalarE, rest on DVE
    se_tiles = []
    acc_ch = 0
    for i, (_, tt) in enumerate(splits):
        se_tiles.append(True)  # BISECT: all ScalarE
        acc_ch += tt

    nc = bacc.Bacc("TRN2", target_bir_lowering=False)
    feat_h = nc.dram_tensor("feat", [Npad, D], bf16, kind="ExternalInput")
    tgt_h = nc.dram_tensor("tgt", [Npad], i32, kind="ExternalInput")
    cen_h = nc.dram_tensor("centers", [C, D], f32, kind="ExternalInput")
    out_h = nc.dram_tensor("out", [1, 8], f32, kind="ExternalOutput")

    # point n == (p, t) with n = p*T + t  -> per-partition contiguous DMA
    featv = feat_h[:, :].rearrange("(p t) d -> p t d", p=P)  # [128, T, 64]
    tgtv = tgt_h[:].rearrange("(p t) -> p t", p=P)  # [128, T]

    with tile.TileContext(nc) as tc:
        with (
            tc.tile_pool(name="consts", bufs=1) as cp,
            tc.tile_pool(name="oh", bufs=3) as ohp,
            tc.tile_pool(name="acc", bufs=1, space="PSUM") as psa,
            tc.tile_pool(name="ps2", bufs=1, space="PSUM") as ps2,
            tc.tile_pool(name="fin", bufs=1) as fp,
        ):
            # ---- constants ----
            iota_rep = cp.tile([P, TT, C], i32)
            nc.gpsimd.iota(
                iota_rep[:, :, :], pattern=[[0, TT], [1, C]], base=0,
                channel_multiplier=0,
            )
            tgt_sb = cp.tile([P, T], i32)
            nc.scalar.dma_start(out=tgt_sb[:, :], in_=tgtv[:, :])
            cen_sb = cp.tile([C, D], f32)
            nc.scalar.dma_start(out=cen_sb[:, :], in_=cen_h[:, :])
            ones = cp.tile([P, 1], f32)
            nc.vector.memset(ones[:, :], 1.0)
            warm = cp.tile([1, 1], f32)
            nc.scalar.activation(
                out=warm[:, :], in_=ones[0:1, :],
                func=mybir.ActivationFunctionType.Sqrt,
            )
            ident = cp.tile([C, C], f32)
            nc.vector.memset(ident[:, :], 1.0)
            nc.gpsimd.affine_select(
                out=ident[:, :], in_=ident[:, :],
                compare_op=mybir.AluOpType.is_equal, fill=0.0,
                base=0, pattern=[[-1, C]], channel_multiplier=1,
            )
            bigeye = cp.tile([1, C, C], f32)
            nc.vector.memset(bigeye[:, :, :], 1e6)
            nc.gpsimd.affine_select(
                out=bigeye[:, :, :], in_=bigeye[:, :, :],
                compare_op=mybir.AluOpType.is_equal, fill=0.0,
                base=0, pattern=[[1, C], [-1, C]], channel_multiplier=0,
            )
            sq_acc = cp.tile([P, ntiles], f32)
            # identity rows at partitions [GRP, GRP+C): lhsT for merging
            # the second column-group accumulator
            GRP = 32
            eye_b = cp.tile([GRP + C, C], f32)
            nc.vector.memset(eye_b[:, :], 1.0)
            nc.gpsimd.affine_select(
                out=eye_b[:, :], in_=eye_b[:, :],
                compare_op=mybir.AluOpType.is_equal, fill=0.0,
                base=-GRP, pattern=[[-1, C]], channel_multiplier=1,
            )
            # explicit ext buffers: [feat | 1] layout; col D memset once
            ext_bufs = [
                cp.tile([P, TT, D + 1], bf16, name=f"ext{j}")
                for j in range(NBUF)
            ]
            for eb in ext_bufs:
                nc.vector.memset(eb[:, :, D : D + 1], 1.0)
            # scratch for the elementwise square outputs (reused)
            se_scr = cp.tile([P, TT, D], bf16)
            ve_scr = cp.tile([P, TT, D], bf16)

            # ---- main loop: accumulate OUT[13, 65] over all points ----
            # two accumulators in different PE column groups so each
            # chunk's LDWEIGHTS overlaps the other group's MATMUL
            acc0 = psa.tile([C, D + 1], f32)
            acc1 = psa.tile([GRP + C, D + 1], f32)
            accs = [acc0[:, :], acc1[GRP : GRP + C, :]]
            last_step = [-1, -1]
            s = 0
            for _, tt in splits:
                for t in range(tt):
                    last_step[s % 2] = s
                    s += 1
            step = 0
            started = [False, False]
            for i, (t0, tt) in enumerate(splits):
                ext = ext_bufs[i % NBUF]
                nc.sync.dma_start(
                    out=ext[:, :tt, 0:D], in_=featv[:, t0 : t0 + tt, :]
                )
                oh = ohp.tile([P, TT, C], bf16, tag="oh")
                nc.vector.tensor_tensor(
                    out=oh[:, :tt, :],
                    in0=tgt_sb[:, t0 : t0 + tt].unsqueeze(2).to_broadcast(
                        [P, tt, C]
                    ),
                    in1=iota_rep[:, :tt, :],
                    op=mybir.AluOpType.is_equal,
                )
                if se_tiles[i]:
                    nc.scalar.activation(
                        out=se_scr[:, :tt, :], in_=ext[:, :tt, 0:D],
                        func=mybir.ActivationFunctionType.Square,
                        accum_out=sq_acc[:, i : i + 1],
                    )
                else:
                    nc.vector.tensor_tensor_reduce(
                        out=ve_scr[:, :tt, :],
                        in0=ext[:, :tt, 0:D],
                        in1=ext[:, :tt, 0:D],
                        scale=1.0,
                        scalar=0.0,
                        op0=mybir.AluOpType.mult,
                        op1=mybir.AluOpType.add,
                        accum_out=sq_acc[:, i : i + 1],
                    )
                for t in range(tt):
                    g = step % 2
                    nc.tensor.matmul(
                        accs[g],
                        lhsT=oh[:, t, :],
                        rhs=ext[:, t, :],
                        start=not started[g],
                        stop=(step == last_step[g]),
                        tile_position=(0, g * GRP),
                    )
                    started[g] = True
                    step += 1

            # ---- finale (tiny, per-core) ----
            c0 = fp.tile([C, D + 1], f32)
            nc.vector.tensor_copy(c0[:, :], acc0[:, :])
            c1 = fp.tile([GRP + C, D + 1], f32)
            nc.vector.tensor_copy(
                c1[GRP : GRP + C, :], acc1[GRP : GRP + C, :]
            )
            acc = ps2.tile([C, D + 1], f32)
            nc.tensor.matmul(
                acc[:, :], lhsT=ident[:, :], rhs=c0[:, :],
                start=True, stop=False,
            )
            nc.tensor.matmul(
                acc[:, :], lhsT=eye_b[GRP : GRP + C, :],
                rhs=c1[GRP : GRP + C, :], start=False, stop=True,
            )
            fsum = acc[:, 0:D]  # [13, 64]
            ccnt = acc[:, D : D + 1]  # [13, 1]

            # per-class means and presence
            cmax = fp.tile([C, 1], f32)
            nc.vector.tensor_scalar(
                out=cmax[:, :], in0=ccnt, scalar1=1.0, scalar2=None,
                op0=mybir.AluOpType.max,
            )
            rec = fp.tile([C, 1], f32)
            nc.vector.reciprocal(rec[:, :], cmax[:, :])
            trin = fp.tile([C, D], f32)
            nc.vector.tensor_scalar(
                out=trin[:, :], in0=fsum, scalar1=rec[:, :], scalar2=None,
                op0=mybir.AluOpType.mult,
            )
            pres = fp.tile([C, 1], f32)
            nc.vector.tensor_scalar(
                out=pres[:, :], in0=ccnt, scalar1=0.0,
                scalar2=None, op0=mybir.AluOpType.is_gt,
            )

            # per-class dot(centers, fsum), ccnt*||c||^2  -> pack3
            junk0 = fp.tile([C, D], f32)
            cn2 = fp.tile([C, 1], f32)
            nc.vector.tensor_tensor(
                out=junk0[:, :], in0=cen_sb[:, :], in1=cen_sb[:, :],
                op=mybir.AluOpType.mult,
            )
            nc.vector.tensor_reduce(
                out=cn2[:, :], in_=junk0[:, :],
                axis=mybir.AxisListType.X, op=mybir.AluOpType.add,
            )
            pack3 = fp.tile([C, 3], f32)
            junk1 = fp.tile([C, D], f32)
            nc.vector.tensor_tensor(
                out=junk1[:, :], in0=cen_sb[:, :], in1=fsum,
                op=mybir.AluOpType.mult,
            )
            nc.vector.tensor_reduce(
                out=pack3[:, 0:1], in_=junk1[:, :],
                axis=mybir.AxisListType.X, op=mybir.AluOpType.add,
            )
            nc.vector.tensor_tensor(
                out=pack3[:, 1:2], in0=cn2[:, :], in1=ccnt,
                op=mybir.AluOpType.mult,
            )
            nc.vector.tensor_copy(pack3[:, 2:3], ccnt)

            # cross-partition sums over the 13 classes: [Tdot, Utot, cnt_b]
            red3 = ps2.tile([1, 3], f32)
            nc.tensor.matmul(
                red3[:, :], lhsT=ones[0:C, :], rhs=pack3[:, :],
                start=True, stop=True,
            )

            # S = sum over all partitions/tiles of sq_acc
            red_sq = fp.tile([P, 1], f32)
            nc.vector.tensor_reduce(
                out=red_sq[:, :], in_=sq_acc[:, :],
                axis=mybir.AxisListType.X, op=mybir.AluOpType.add,
            )
            s_ps = ps2.tile([1, 1], f32)
            nc.tensor.matmul(
                s_ps[:, :], lhsT=ones[:, :], rhs=red_sq[:, :],
                start=True, stop=True,
            )

            # transpose cmeans -> [64, 13]; present -> [1, 13]
            trps = ps2.tile([D, C], f32)
            nc.tensor.transpose(trps[:, :], trin[:, :], ident[:, :])
            trsb = fp.tile([D, C], f32)
            nc.vector.tensor_copy(trsb[:, :], trps[:, :])
            cmT = trsb[0:D, :]  # [64, 13]
            prps = ps2.tile([1, C], f32)
            nc.tensor.transpose(prps[:, :], pres[:, :], ident[:, :])
            presT = fp.tile([1, C], f32)
            nc.vector.tensor_copy(presT[:, :], prps[:, :])

            # pairwise squared distances between class means
            diff = fp.tile([D, C, C], f32)
            nc.vector.tensor_tensor(
                out=diff[:, :, :],
                in0=cmT.unsqueeze(2).to_broadcast([D, C, C]),
                in1=cmT.unsqueeze(1).to_broadcast([D, C, C]),
                op=mybir.AluOpType.subtract,
            )
            dsq = fp.tile([D, C, C], f32)
            nc.vector.tensor_tensor(
                out=dsq[:, :, :], in0=diff[:, :, :], in1=diff[:, :, :],
                op=mybir.AluOpType.mult,
            )
            dd2 = ps2.tile([1, C * C], f32)
            nc.tensor.matmul(
                dd2[:, :], lhsT=ones[0:1, :],
                rhs=bigeye[:, :, :].rearrange("p a b -> p (a b)"),
                start=True, stop=False,
            )
            nc.tensor.matmul(
                dd2[:, :], lhsT=ones[0:D, :],
                rhs=dsq[:, :, :].rearrange("d a b -> d (a b)"),
                start=False, stop=True,
            )
            dist = fp.tile([1, C * C], f32)
            nc.scalar.activation(
                out=dist[:, :], in_=dd2[:, :],
                func=mybir.ActivationFunctionType.Sqrt,
            )
            hinge = fp.tile([1, C * C], f32)  # holds -relu(M - dist)
            nc.vector.tensor_scalar(
                out=hinge[:, :], in0=dist[:, :], scalar1=MARGIN,
                scalar2=MARGIN, op0=mybir.AluOpType.min,
                op1=mybir.AluOpType.subtract,
            )
            pm = fp.tile([1, C, C], f32)
            nc.vector.tensor_tensor(
                out=pm[:, :, :],
                in0=presT[:, :].unsqueeze(2).to_broadcast([1, C, C]),
                in1=presT[:, :].unsqueeze(1).to_broadcast([1, C, C]),
                op=mybir.AluOpType.mult,
            )
            pmf = pm[:, :, :].rearrange("p a b -> p (a b)")
            # raw per-batch sums; host does the final few divisions
            scal = fp.tile([1, 8], f32)
            nc.vector.memset(scal[:, 6:8], 0.0)
            terms = fp.tile([1, C * C], f32)
            nc.vector.tensor_tensor(
                out=terms[:, :], in0=hinge[:, :], in1=pmf,
                op=mybir.AluOpType.mult,
            )
            nc.vector.tensor_reduce(
                out=scal[:, 4:5], in_=terms[:, :],
                axis=mybir.AxisListType.X, op=mybir.AluOpType.add,
            )
            nc.vector.tensor_reduce(
                out=scal[:, 5:6], in_=presT[:, :], axis=mybir.AxisListType.X,
                op=mybir.AluOpType.add,
            )
            nc.vector.tensor_copy(scal[:, 0:1], s_ps[:, :])
            nc.vector.tensor_copy(scal[:, 1:4], red3[:, :])

            nc.sync.dma_start(out=out_h[:, :], in_=scal[:, :])
    nc.finalize()
    return nc


# set by test.py to capture profile info
TRACE = False
LAST = {}


def _ensure_ntff_hook():
    """The agent image's antenv lacks axon_hooks; synthesize it so
    run_bass_kernel_spmd(trace=True) can profile. Best-effort."""
    import sys
    import types

    try:
        from antenv.axon_hooks import get_axon_ntff_profile_hook  # noqa: F401
        return
    except ImportError:
        pass
    try:
        from trn_agent_boot.trn_boot import _ntff_profile_via_ctypes

        hook = _ntff_profile_via_ctypes("/opt/axon/libaxon_pjrt.so")
        mod = types.ModuleType("antenv.axon_hooks")
        mod._hook = hook
        mod.get_axon_ntff_profile_hook = lambda: mod._hook
        mod.set_axon_ntff_profile_hook = lambda h: setattr(mod, "_hook", h)
        sys.modules["antenv.axon_hooks"] = mod
        import antenv

        antenv.axon_hooks = mod
    except Exception as e:  # degrade: no profile, run still works
        print(f"ntff hook injection failed: {e}")


def kernel(pred=None, target=None, feat=None, batch=None, centers=None):
    target = np.asarray(target)
    feat = np.asarray(feat, dtype=np.float32)
    batch = np.asarray(batch)
    centers = np.asarray(centers, dtype=np.float32)
    N = feat.shape[0]

    # shard at batch boundaries: core b <- batch b (batch is sorted)
    bounds = np.searchsorted(batch, np.arange(B + 1))
    sizes = np.diff(bounds)
    T = int(max((int(sizes.max()) + P - 1) // P, TT))
    Npad = P * T

    in_maps = []
    for b in range(B):
        lo, hi = int(bounds[b]), int(bounds[b + 1])
        fb = np.zeros((Npad, D), dtype=ml_dtypes.bfloat16)
        tb = np.full((Npad,), C, dtype=np.int32)
        fb[: hi - lo] = feat[lo:hi].astype(ml_dtypes.bfloat16)
        tb[: hi - lo] = target[lo:hi]
        inv = tb == IGNORE
        if inv.any():
            tb[inv] = C  # one-hot miss -> excluded everywhere
            fb[inv] = 0.0  # excluded from S
        in_maps.append({"feat": fb, "tgt": tb, "centers": centers})

    nc = build_nc(T)
    if TRACE:
        _ensure_ntff_hook()
    res = run_bass_kernel_spmd(nc, in_maps, list(range(N_CORES)), trace=TRACE)
    LAST["results"] = res

    rows = np.stack(
        [np.asarray(res.results[b]["out"]).reshape(8) for b in range(B)]
    ).astype(np.float64)
    s, tdot, utot, cnt_b, tsum, kpres = (rows[:, j] for j in range(6))
    npairs = kpres * (kpres - 1.0)
    intra = (s - 2.0 * tdot + utot) / np.maximum(cnt_b, 1.0)
    inter = -tsum / np.maximum(npairs, 1.0)
    present = cnt_b > 0
    den = max(float(present.sum()), 1.0)
    loss = LOSS_W * (
        INTRA_W * float(np.where(present, intra, 0.0).sum()) / den
        + INTER_W * float(np.where(present, inter, 0.0).sum()) / den
    )
    return np.float32(loss)


# revision 11
# speedup vs baseline: 1.5659x; 1.5659x over previous
"""DynamicCenterLoss on Trainium2 (Bass/Tile), 8-core SPMD.

Strategy: `batch` is sorted, so shard at batch boundaries -> core b owns
batch b (sizes ~N/8 +- <1%). feat is cast to bf16 on the host, halving
HBM traffic (the PE consumed bf16 anyway); the DMA lands dense
(contiguous per partition).  Per core, per 128-point chunk the PE
computes two matmuls off one weights load (oh14 = [onehot | 1]):

    stats[14, 64] += oh14^T @ feat   -> rows 0..12: per-class sums
    cnt[14, 14]   += oh14^T @ oh14   -> row 13, cols 0..12: class counts

S = sum_n ||feat_n||^2 is split between the Scalar engine (activation
Square + accum) and the Vector engine (bn_stats: sum from count*var +
count*mean^2) so no engine exceeds the DMA floor.  The intra term uses
sum_n ||f_n - c_{t_n}||^2 = S - 2*sum_c c_c.fsum_c + sum_c
ccnt_c*||c_c||^2, so no per-point gather of centers is ever needed.
Padded rows (target=13) produce a zero one-hot row (cols 0..12) and
zero features, so they contribute nothing. The pairwise-center hinge
loss is computed per core on its own (13,64) stats; the host only
averages the 8 per-batch scalars.
"""

import numpy as np
import ml_dtypes

import concourse.bass as bass
import concourse.bacc as bacc
import concourse.tile as tile
from concourse import mybir
from concourse.bass_utils import run_bass_kernel_spmd

P = 128
D = 64
C = 13
B = 8
N_CORES = 8
MARGIN = 0.5
INTRA_W = 1.0
INTER_W = 1.0
LOSS_W = 0.01
IGNORE = -1
TT = 64  # matmul steps (128-point chunks) per SBUF tile
NBUF = 6  # feat buffers
NOBUF = 3  # oh14 buffers (ones column memset once)
BNG = 8  # bn_stats groups per tile (512 elems each)

f32 = mybir.dt.float32
bf16 = mybir.dt.bfloat16
i32 = mybir.dt.int32


def build_nc(T: int) -> bass.Bass:
    """Build the per-core Bass program. T = points per SBUF partition."""
    Npad = P * T
    # tile splits: small first tiles so the PE pipeline fills early
    splits = []
    _t0 = 0
    szs = [16, 48]
    while sum(szs) + TT <= T:
        szs.append(TT)
    for sz in szs:
        if _t0 >= T:
            break
        sz = min(sz, T - _t0)
        splits.append((_t0, sz))
        _t0 += sz
    if _t0 < T:
        splits.append((_t0, T - _t0))
    ntiles = len(splits)
    # square-pass split: ~62% of chunks on ScalarE; trailing full tiles
    # (tt == TT) go to DVE bn_stats
    se_tiles = []
    acc_ch = 0
    for i, (_, tt) in enumerate(splits):
        se_tiles.append(acc_ch < int(0.62 * T) or tt != TT)
        acc_ch += tt
    dve_tiles = [i for i in range(ntiles) if not se_tiles[i]]
    n_dve = max(len(dve_tiles), 1)

    nc = bacc.Bacc("TRN2", target_bir_lowering=False)
    feat_h = nc.dram_tensor("feat", [Npad, D], bf16, kind="ExternalInput")
    tgt_h = nc.dram_tensor("tgt", [Npad], i32, kind="ExternalInput")
    cen_h = nc.dram_tensor("centers", [C, D], f32, kind="ExternalInput")
    out_h = nc.dram_tensor("out", [1, 8], f32, kind="ExternalOutput")

    # point n == (p, t) with n = p*T + t  -> per-partition contiguous DMA
    featv = feat_h[:, :].rearrange("(p t) d -> p t d", p=P)  # [128, T, 64]
    tgtv = tgt_h[:].rearrange("(p t) -> p t", p=P)  # [128, T]

    with tile.TileContext(nc) as tc:
        with (
            tc.tile_pool(name="consts", bufs=1) as cp,
            tc.tile_pool(name="acc", bufs=1, space="PSUM") as psa,
            tc.tile_pool(name="ps2", bufs=1, space="PSUM") as ps2,
            tc.tile_pool(name="fin", bufs=1) as fp,
        ):
            # ---- constants ----
            iota_rep = cp.tile([P, TT, C], i32)
            nc.gpsimd.iota(
                iota_rep[:, :, :], pattern=[[0, TT], [1, C]], base=0,
                channel_multiplier=0,
            )
            tgt_sb = cp.tile([P, T], i32)
            nc.scalar.dma_start(out=tgt_sb[:, :], in_=tgtv[:, :])
            cen_sb = cp.tile([C, D], f32)
            nc.scalar.dma_start(out=cen_sb[:, :], in_=cen_h[:, :])
            ones = cp.tile([P, 1], f32)
            nc.vector.memset(ones[:, :], 1.0)
            warm = cp.tile([1, 1], f32)
            nc.scalar.activation(
                out=warm[:, :], in_=ones[0:1, :],
                func=mybir.ActivationFunctionType.Sqrt,
            )
            ident = cp.tile([C, C], f32)
            nc.vector.memset(ident[:, :], 1.0)
            nc.gpsimd.affine_select(
                out=ident[:, :], in_=ident[:, :],
                compare_op=mybir.AluOpType.is_equal, fill=0.0,
                base=0, pattern=[[-1, C]], channel_multiplier=1,
            )
            ident14 = cp.tile([C + 1, C + 1], f32)
            nc.vector.memset(ident14[:, :], 1.0)
            nc.gpsimd.affine_select(
                out=ident14[:, :], in_=ident14[:, :],
                compare_op=mybir.AluOpType.is_equal, fill=0.0,
                base=0, pattern=[[-1, C + 1]], channel_multiplier=1,
            )
            bigeye = cp.tile([1, C, C], f32)
            nc.vector.memset(bigeye[:, :, :], 1e6)
            nc.gpsimd.affine_select(
                out=bigeye[:, :, :], in_=bigeye[:, :, :],
                compare_op=mybir.AluOpType.is_equal, fill=0.0,
                base=0, pattern=[[1, C], [-1, C]], channel_multiplier=0,
            )
            sq_acc = cp.tile([P, ntiles], f32)
            nc.vector.memset(sq_acc[:, :], 0.0)
            # identity rows at partitions [GRP, GRP+14): lhsT for merging
            # the second column-group accumulator
            GRP = 32
            eye_b = cp.tile([GRP + C + 1, C + 1], f32)
            nc.vector.memset(eye_b[:, :], 1.0)
            nc.gpsimd.affine_select(
                out=eye_b[:, :], in_=eye_b[:, :],
                compare_op=mybir.AluOpType.is_equal, fill=0.0,
                base=-GRP, pattern=[[-1, C + 1]], channel_multiplier=1,
            )
            # explicit buffers; oh14 col 13 (ones) memset once
            fbufs = [
                cp.tile([P, TT, D], bf16, name=f"fb{j}") for j in range(NBUF)
            ]
            obufs = [
                cp.tile([P, TT, C + 1], bf16, name=f"ob{j}")
                for j in range(NOBUF)
            ]
            for ob in obufs:
                nc.vector.memset(ob[:, :, C : C + 1], 1.0)
            # scratch for ScalarE square outputs; bn_stats outputs
            se_scr = cp.tile([P, TT, D], bf16)
            bn_out = cp.tile([P, n_dve, BNG, 6], f32)

            # ---- main loop ----
            # two accumulator sets in different PE column groups so each
            # chunk's LDWEIGHTS overlaps the other group's MATMUL
            st0 = psa.tile([C + 1, D], f32)
            st1 = psa.tile([GRP + C + 1, D], f32)
            cn0 = psa.tile([C + 1, C + 1], f32)
            cn1 = psa.tile([GRP + C + 1, C + 1], f32)
            sts = [st0[:, :], st1[GRP : GRP + C + 1, :]]
            cns = [cn0[:, :], cn1[GRP : GRP + C + 1, :]]
            last_step = [-1, -1]
            s = 0
            for _, tt in splits:
                for t in range(tt):
                    last_step[s % 2] = s
                    s += 1
            step = 0
            started = [False, False]
            for i, (t0, tt) in enumerate(splits):
                fb = fbufs[i % NBUF]
                nc.sync.dma_start(
                    out=fb[:, :tt, :], in_=featv[:, t0 : t0 + tt, :]
                )
                oh = obufs[i % NOBUF]
                nc.vector.tensor_tensor(
                    out=oh[:, :tt, 0:C],
                    in0=tgt_sb[:, t0 : t0 + tt].unsqueeze(2).to_broadcast(
                        [P, tt, C]
                    ),
                    in1=iota_rep[:, :tt, :],
                    op=mybir.AluOpType.is_equal,
                )
                if se_tiles[i]:
                    nc.scalar.activation(
                        out=se_scr[:, :tt, :], in_=fb[:, :tt, :],
                        func=mybir.ActivationFunctionType.Square,
                        accum_out=sq_acc[:, i : i + 1],
                    )
                else:
                    di = dve_tiles.index(i)
                    fbg = fb[:, :, :].rearrange("p t d -> p (t d)").rearrange(
                        "p (g f) -> p g f", g=BNG
                    )
                    for bg in range(BNG):
                        nc.vector.bn_stats(
                            out=bn_out[:, di, bg, :],
                            in_=fbg[:, bg, :],
                        )
                for t in range(tt):
                    g = step % 2
                    nc.tensor.matmul(
                        sts[g],
                        lhsT=oh[:, t, :],
                        rhs=fb[:, t, :],
                        start=not started[g],
                        stop=(step == last_step[g]),
                        tile_position=(0, g * GRP),
                    )
                    nc.tensor.matmul(
                        cns[g],
                        lhsT=oh[:, t, :],
                        rhs=oh[:, t, :],
                        start=not started[g],
                        stop=(step == last_step[g]),
                        tile_position=(0, g * GRP),
                        skip_group_check=True,
                    )
                    started[g] = True
                    step += 1

            # ---- finale (tiny, per-core) ----
            # pack stats+cnt into [14, 78], merge the two column groups
            c0 = fp.tile([C + 1, D + C + 1], f32)
            nc.vector.tensor_copy(c0[:, 0:D], st0[:, :])
            nc.vector.tensor_copy(c0[:, D : D + C + 1], cn0[:, :])
            c1 = fp.tile([GRP + C + 1, D + C + 1], f32)
            nc.vector.tensor_copy(
                c1[GRP : GRP + C + 1, 0:D], st1[GRP : GRP + C + 1, :]
            )
            nc.vector.tensor_copy(
                c1[GRP : GRP + C + 1, D : D + C + 1],
                cn1[GRP : GRP + C + 1, :],
            )
            acc = ps2.tile([C + 1, D + C + 1], f32)
            nc.tensor.matmul(
                acc[:, :], lhsT=ident14[:, :], rhs=c0[:, :],
                start=True, stop=False,
            )
            nc.tensor.matmul(
                acc[:, :], lhsT=eye_b[GRP : GRP + C + 1, :],
                rhs=c1[GRP : GRP + C + 1, :], start=False, stop=True,
            )
            # shared PSUM scratch for the small finale matmuls
            trps = ps2.tile([D, C], f32)
            misc = ps2.tile([C + 1, 200], f32)
            # cnt matrix is symmetric: counts also live in col 13 of the
            # cnt block (rows 0..12) -> partition-0-aligned read
            ccnt = fp.tile([C, 1], f32)
            nc.vector.tensor_copy(ccnt[:, :], acc[0:C, D + C : D + C + 1])
            fsum_sb = fp.tile([C, D], f32)
            nc.vector.tensor_copy(fsum_sb[:, :], acc[0:C, 0:D])

            # per-class means and presence
            cmax = fp.tile([C, 1], f32)
            nc.vector.tensor_scalar(
                out=cmax[:, :], in0=ccnt[:, :], scalar1=1.0, scalar2=None,
                op0=mybir.AluOpType.max,
            )
            rec = fp.tile([C, 1], f32)
            nc.vector.reciprocal(rec[:, :], cmax[:, :])
            trin = fp.tile([C, D], f32)
            nc.vector.tensor_scalar(
                out=trin[:, :], in0=fsum_sb[:, :], scalar1=rec[:, :],
                scalar2=None, op0=mybir.AluOpType.mult,
            )
            pres = fp.tile([C, 1], f32)
            nc.vector.tensor_scalar(
                out=pres[:, :], in0=ccnt[:, :], scalar1=0.0,
                scalar2=None, op0=mybir.AluOpType.is_gt,
            )

            # per-class dot(centers, fsum), ccnt*||c||^2  -> pack3
            junk0 = fp.tile([C, D], f32)
            cn2 = fp.tile([C, 1], f32)
            nc.vector.tensor_tensor(
                out=junk0[:, :], in0=cen_sb[:, :], in1=cen_sb[:, :],
                op=mybir.AluOpType.mult,
            )
            nc.vector.tensor_reduce(
                out=cn2[:, :], in_=junk0[:, :],
                axis=mybir.AxisListType.X, op=mybir.AluOpType.add,
            )
            pack3 = fp.tile([C, 3], f32)
            junk1 = fp.tile([C, D], f32)
            nc.vector.tensor_tensor(
                out=junk1[:, :], in0=cen_sb[:, :], in1=fsum_sb[:, :],
                op=mybir.AluOpType.mult,
            )
            nc.vector.tensor_reduce(
                out=pack3[:, 0:1], in_=junk1[:, :],
                axis=mybir.AxisListType.X, op=mybir.AluOpType.add,
            )
            nc.vector.tensor_tensor(
                out=pack3[:, 1:2], in0=cn2[:, :], in1=ccnt[:, :],
                op=mybir.AluOpType.mult,
            )
            nc.vector.tensor_copy(pack3[:, 2:3], ccnt[:, :])

            # cross-partition sums over the 13 classes: [Tdot, Utot, cnt_b]
            red3 = misc[0:1, 4:7]
            nc.tensor.matmul(
                red3, lhsT=ones[0:C, :], rhs=pack3[:, :],
                start=True, stop=True,
            )

            # S: ScalarE part from sq_acc; DVE part from bn_out:
            # sum_g [cv_e + c_e*m_e^2 + cv_o + c_o*m_o^2]
            bn_flat = bn_out[:, :, :, :].rearrange("p a g s -> p (a g) s")
            nbg = n_dve * BNG
            m2 = fp.tile([P, nbg, 2], f32)
            nc.vector.tensor_tensor(
                out=m2[:, :, 0:1], in0=bn_flat[:, :, 1:2],
                in1=bn_flat[:, :, 1:2], op=mybir.AluOpType.mult,
            )
            nc.vector.tensor_tensor(
                out=m2[:, :, 1:2], in0=bn_flat[:, :, 4:5],
                in1=bn_flat[:, :, 4:5], op=mybir.AluOpType.mult,
            )
            cm2 = fp.tile([P, nbg, 2], f32)
            nc.vector.tensor_tensor(
                out=cm2[:, :, 0:1], in0=m2[:, :, 0:1],
                in1=bn_flat[:, :, 0:1], op=mybir.AluOpType.mult,
            )
            nc.vector.tensor_tensor(
                out=cm2[:, :, 1:2], in0=m2[:, :, 1:2],
                in1=bn_flat[:, :, 3:4], op=mybir.AluOpType.mult,
            )
            sqv = fp.tile([P, nbg, 2], f32)
            nc.vector.tensor_tensor(
                out=sqv[:, :, 0:1], in0=cm2[:, :, 0:1],
                in1=bn_flat[:, :, 2:3], op=mybir.AluOpType.add,
            )
            nc.vector.tensor_tensor(
                out=sqv[:, :, 1:2], in0=cm2[:, :, 1:2],
                in1=bn_flat[:, :, 5:6], op=mybir.AluOpType.add,
            )
            red_bn = fp.tile([P, 1], f32)
            nc.vector.tensor_reduce(
                out=red_bn[:, :],
                in_=sqv[:, :, :].rearrange("p g s -> p (g s)"),
                axis=mybir.AxisListType.X, op=mybir.AluOpType.add,
            )
            red_sq = fp.tile([P, 1], f32)
            nc.vector.tensor_reduce(
                out=red_sq[:, :], in_=sq_acc[:, :],
                axis=mybir.AxisListType.X, op=mybir.AluOpType.add,
            )
            red_all = fp.tile([P, 1], f32)
            nc.vector.tensor_tensor(
                out=red_all[:, :], in0=red_sq[:, :], in1=red_bn[:, :],
                op=mybir.AluOpType.add,
            )
            s_ps = misc[0:1, 8:9]
            nc.tensor.matmul(
                s_ps, lhsT=ones[:, :], rhs=red_all[:, :],
                start=True, stop=True,
            )

            # transpose cmeans -> [64, 13]; present -> [1, 13]
            nc.tensor.transpose(trps[:, :], trin[:, :], ident[:, :])
            trsb = fp.tile([D, C], f32)
            nc.vector.tensor_copy(trsb[:, :], trps[:, :])
            cmT = trsb[0:D, :]  # [64, 13]
            prps = misc[0:1, 12:12 + C]
            nc.tensor.transpose(prps, pres[:, :], ident[:, :])
            presT = fp.tile([1, C], f32)
            nc.vector.tensor_copy(presT[:, :], prps)

            # pairwise squared distances between class means
            diff = fp.tile([D, C, C], f32)
            nc.vector.tensor_tensor(
                out=diff[:, :, :],
                in0=cmT.unsqueeze(2).to_broadcast([D, C, C]),
                in1=cmT.unsqueeze(1).to_broadcast([D, C, C]),
                op=mybir.AluOpType.subtract,
            )
            dsq = fp.tile([D, C, C], f32)
            nc.vector.tensor_tensor(
                out=dsq[:, :, :], in0=diff[:, :, :], in1=diff[:, :, :],
                op=mybir.AluOpType.mult,
            )
            dd2 = misc[0:1, 28:28 + C * C]
            nc.tensor.matmul(
                dd2, lhsT=ones[0:1, :],
                rhs=bigeye[:, :, :].rearrange("p a b -> p (a b)"),
                start=True, stop=False,
            )
            nc.tensor.matmul(
                dd2, lhsT=ones[0:D, :],
                rhs=dsq[:, :, :].rearrange("d a b -> d (a b)"),
                start=False, stop=True,
            )
            dist = fp.tile([1, C * C], f32)
            nc.scalar.activation(
                out=dist[:, :], in_=dd2,
                func=mybir.ActivationFunctionType.Sqrt,
            )
            hinge = fp.tile([1, C * C], f32)  # holds -relu(M - dist)
            nc.vector.tensor_scalar(
                out=hinge[:, :], in0=dist[:, :], scalar1=MARGIN,
                scalar2=MARGIN, op0=mybir.AluOpType.min,
                op1=mybir.AluOpType.subtract,
            )
            pm = fp.tile([1, C, C], f32)
            nc.vector.tensor_tensor(
                out=pm[:, :, :],
                in0=presT[:, :].unsqueeze(2).to_broadcast([1, C, C]),
                in1=presT[:, :].unsqueeze(1).to_broadcast([1, C, C]),
                op=mybir.AluOpType.mult,
            )
            pmf = pm[:, :, :].rearrange("p a b -> p (a b)")
            # raw per-batch sums; host does the final few divisions
            scal = fp.tile([1, 8], f32)
            nc.vector.memset(scal[:, 6:8], 0.0)
            terms = fp.tile([1, C * C], f32)
            nc.vector.tensor_tensor(
                out=terms[:, :], in0=hinge[:, :], in1=pmf,
                op=mybir.AluOpType.mult,
            )
            nc.vector.tensor_reduce(
                out=scal[:, 4:5], in_=terms[:, :],
                axis=mybir.AxisListType.X, op=mybir.AluOpType.add,
            )
            nc.vector.tensor_reduce(
                out=scal[:, 5:6], in_=presT[:, :], axis=mybir.AxisListType.X,
                op=mybir.AluOpType.add,
            )
            nc.vector.tensor_copy(scal[:, 0:1], s_ps)
            nc.vector.tensor_copy(scal[:, 1:4], red3)

            nc.sync.dma_start(out=out_h[:, :], in_=scal[:, :])
    nc.finalize()
    return nc


# set by test.py to capture profile info
TRACE = False
LAST = {}


def _ensure_ntff_hook():
    """The agent image's antenv lacks axon_hooks; synthesize it so
    run_bass_kernel_spmd(trace=True) can profile. Best-effort."""
    import sys
    import types

    try:
        from antenv.axon_hooks import get_axon_ntff_profile_hook  # noqa: F401
        return
    except ImportError:
        pass
    try:
        from trn_agent_boot.trn_boot import _ntff_profile_via_ctypes

        hook = _ntff_profile_via_ctypes("/opt/axon/libaxon_pjrt.so")
        mod = types.ModuleType("antenv.axon_hooks")
        mod._hook = hook
        mod.get_axon_ntff_profile_hook = lambda: mod._hook
        mod.set_axon_ntff_profile_hook = lambda h: setattr(mod, "_hook", h)
        sys.modules["antenv.axon_hooks"] = mod
        import antenv

        antenv.axon_hooks = mod
    except Exception as e:  # degrade: no profile, run still works
        print(f"ntff hook injection failed: {e}")


def kernel(pred=None, target=None, feat=None, batch=None, centers=None):
    target = np.asarray(target)
    feat = np.asarray(feat, dtype=np.float32)
    batch = np.asarray(batch)
    centers = np.asarray(centers, dtype=np.float32)
    N = feat.shape[0]

    # shard at batch boundaries: core b <- batch b (batch is sorted)
    bounds = np.searchsorted(batch, np.arange(B + 1))
    sizes = np.diff(bounds)
    T = int(max((int(sizes.max()) + P - 1) // P, TT))
    Npad = P * T

    in_maps = []
    for b in range(B):
        lo, hi = int(bounds[b]), int(bounds[b + 1])
        fb = np.zeros((Npad, D), dtype=ml_dtypes.bfloat16)
        tb = np.full((Npad,), C, dtype=np.int32)
        fb[: hi - lo] = feat[lo:hi].astype(ml_dtypes.bfloat16)
        tb[: hi - lo] = target[lo:hi]
        inv = tb == IGNORE
        if inv.any():
            tb[inv] = C  # one-hot miss -> excluded everywhere
            fb[inv] = 0.0  # excluded from S
        in_maps.append({"feat": fb, "tgt": tb, "centers": centers})

    nc = build_nc(T)
    if TRACE:
        _ensure_ntff_hook()
    res = run_bass_kernel_spmd(nc, in_maps, list(range(N_CORES)), trace=TRACE)
    LAST["results"] = res

    rows = np.stack(
        [np.asarray(res.results[b]["out"]).reshape(8) for b in range(B)]
    ).astype(np.float64)
    s, tdot, utot, cnt_b, tsum, kpres = (rows[:, j] for j in range(6))
    npairs = kpres * (kpres - 1.0)
    intra = (s - 2.0 * tdot + utot) / np.maximum(cnt_b, 1.0)
    inter = -tsum / np.maximum(npairs, 1.0)
    present = cnt_b > 0
    den = max(float(present.sum()), 1.0)
    loss = LOSS_W * (
        INTRA_W * float(np.where(present, intra, 0.0).sum()) / den
        + INTER_W * float(np.where(present, inter, 0.0).sum()) / den
    )
    return np.float32(loss)


# revision 15
# speedup vs baseline: 2.0911x; 1.3354x over previous
"""DynamicCenterLoss on Trainium2 (Bass/Tile), 8-core SPMD.

Strategy: `batch` is sorted, so shard at batch boundaries -> core b owns
batch b (sizes ~N/8 +- <1%). The host ships feat as bf16 with a ones
column appended ([feat | 1], 65 cols) so the DMA lands dense and the
single per-chunk matmul yields per-class feature sums AND counts:

    OUT[13, 65] += onehot^T @ [feat | 1]

The matmul accumulates into four PE column groups: (SE-even, SE-odd,
BN-even, BN-odd), where SE/BN tags which engine computes that chunk's
sum-of-squares share: the Scalar engine (activation Square + accum,
reading the strided feat columns) or the Vector engine (bn_stats over
flat 512-element runs that INCLUDE the ones column; the finale
subtracts the BN groups' valid count, which the BN accumulators
provide exactly).  The one-hot is built class-major with int16
target/iota so the DVE runs in 2x mode.  The intra term uses
sum_n ||f_n - c_{t_n}||^2 = S - 2*sum_c c_c.fsum_c + sum_c
ccnt_c*||c_c||^2, so no per-point gather of centers is ever needed.
Padded rows (target=13) are fully zeroed by the host (features AND
ones column) and produce a zero one-hot row, so they contribute
nothing anywhere. The pairwise-center hinge loss is computed per core
on its own (13,64) stats; the host only averages the 8 per-batch
scalars.
"""

import numpy as np
import ml_dtypes

import concourse.bass as bass
import concourse.bacc as bacc
import concourse.tile as tile
from concourse import mybir
from concourse.bass_utils import run_bass_kernel_spmd

P = 128
D = 64
E = D + 1  # feat cols + ones column
C = 13
B = 8
N_CORES = 8
MARGIN = 0.5
INTRA_W = 1.0
INTER_W = 1.0
LOSS_W = 0.01
IGNORE = -1
TT = 64  # matmul steps (128-point chunks) per SBUF tile
NBUF = 6  # ext buffers
NOBUF = 3  # onehot buffers
SE_CH = 352  # chunks whose squares go to ScalarE (rest: DVE bn_stats)

f32 = mybir.dt.float32
bf16 = mybir.dt.bfloat16
i32 = mybir.dt.int32
i16 = mybir.dt.int16


def build_nc(T: int) -> bass.Bass:
    """Build the per-core Bass program. T = points per SBUF partition."""
    Npad = P * T
    # tile splits: small first tiles so the PE pipeline fills early
    splits = []
    _t0 = 0
    szs = [16, 48]
    while sum(szs) + TT <= T:
        szs.append(TT)
    for sz in szs:
        if _t0 >= T:
            break
        sz = min(sz, T - _t0)
        splits.append((_t0, sz))
        _t0 += sz
    if _t0 < T:
        splits.append((_t0, T - _t0))
    ntiles = len(splits)
    se_ch = min(SE_CH, T)

    # per-tile: how many leading chunks are ScalarE; the rest are BN
    se_in_tile = []
    acc_ch = 0
    for i, (_, tt) in enumerate(splits):
        se_in_tile.append(max(0, min(tt, se_ch - acc_ch)))
        acc_ch += tt

    # count bn_stats flat groups per tile (512-elem runs over tt*65)
    bn_groups = []
    for i, (_, tt) in enumerate(splits):
        se_t = se_in_tile[i]
        bn_el = (tt - se_t) * E
        bn_groups.append((bn_el + 511) // 512 if bn_el else 0)
    nbn = sum(bn_groups)

    nc = bacc.Bacc("TRN2", target_bir_lowering=False)
    feat_h = nc.dram_tensor("feat", [Npad, E], bf16, kind="ExternalInput")
    tgt_h = nc.dram_tensor("tgt", [Npad], i16, kind="ExternalInput")
    cen_h = nc.dram_tensor("centers", [C, D], f32, kind="ExternalInput")
    out_h = nc.dram_tensor("out", [1, 8], f32, kind="ExternalOutput")

    # point n == (p, t) with n = p*T + t  -> per-partition contiguous DMA
    featv = feat_h[:, :].rearrange("(p t) e -> p t e", p=P)  # [128, T, 65]
    tgtv = tgt_h[:].rearrange("(p t) -> p t", p=P)  # [128, T]

    with tile.TileContext(nc) as tc:
        with (
            tc.tile_pool(name="consts", bufs=1) as cp,
            tc.tile_pool(name="acc", bufs=1, space="PSUM") as psa,
            tc.tile_pool(name="ps2", bufs=1, space="PSUM") as ps2,
            tc.tile_pool(name="fin", bufs=1) as fp,
        ):
            # ---- constants ----
            iota32 = cp.tile([P, C, TT], i32)
            nc.gpsimd.iota(
                iota32[:, :, :], pattern=[[1, C], [0, TT]], base=0,
                channel_multiplier=0,
            )
            iota_cm = cp.tile([P, C, TT], i16)
            nc.vector.tensor_copy(iota_cm[:, :, :], iota32[:, :, :])
            tgt_sb = cp.tile([P, T], i16)
            nc.scalar.dma_start(out=tgt_sb[:, :], in_=tgtv[:, :])
            cen_sb = cp.tile([C, D], f32)
            nc.scalar.dma_start(out=cen_sb[:, :], in_=cen_h[:, :])
            ones = cp.tile([P, 1], f32)
            nc.vector.memset(ones[:, :], 1.0)
            warm = cp.tile([1, 1], f32)
            nc.scalar.activation(
                out=warm[:, :], in_=ones[0:1, :],
                func=mybir.ActivationFunctionType.Sqrt,
            )
            ident = cp.tile([C, C], f32)
            nc.vector.memset(ident[:, :], 1.0)
            nc.gpsimd.affine_select(
                out=ident[:, :], in_=ident[:, :],
                compare_op=mybir.AluOpType.is_equal, fill=0.0,
                base=0, pattern=[[-1, C]], channel_multiplier=1,
            )
            bigeye = cp.tile([1, C, C], f32)
            nc.vector.memset(bigeye[:, :, :], 1e6)
            nc.gpsimd.affine_select(
                out=bigeye[:, :, :], in_=bigeye[:, :, :],
                compare_op=mybir.AluOpType.is_equal, fill=0.0,
                base=0, pattern=[[1, C], [-1, C]], channel_multiplier=0,
            )
            sq_acc = cp.tile([P, ntiles], f32)
            nc.vector.memset(sq_acc[:, :], 0.0)
            # identity rows at partition offset 32: lhsT for merging
            # the odd column-group accumulators
            eye32 = cp.tile([32 + C, C], f32)
            nc.vector.memset(eye32[:, :], 1.0)
            nc.gpsimd.affine_select(
                out=eye32[:, :], in_=eye32[:, :],
                compare_op=mybir.AluOpType.is_equal, fill=0.0,
                base=-32, pattern=[[-1, C]], channel_multiplier=1,
            )
            eyes = [ident[:, :], eye32[32 : 32 + C, :],
                    ident[:, :], eye32[32 : 32 + C, :]]
            ebufs = [
                cp.tile([P, TT, E], bf16, name=f"eb{j}") for j in range(NBUF)
            ]
            obufs = [
                cp.tile([P, C, TT], bf16, name=f"ob{j}")
                for j in range(NOBUF)
            ]
            # scratch for ScalarE square outputs; bn_stats outputs
            se_scr = cp.tile([P, TT, D], bf16)
            bn_out = cp.tile([P, nbn, 6], f32)

            # ---- main loop: accumulate OUT[13, 65] over all points ----
            # four accumulators: (SE even, SE odd, BN even, BN odd);
            # parity -> PE column group (LDWEIGHTS ping-pong), SE/BN ->
            # distinct PSUM banks so the finale sees the BN valid count
            accs_t = []
            for gi, gofs in enumerate((0, 32, 0, 32)):
                at = psa.tile([gofs + C, E], f32, name=f"pacc{gi}")
                accs_t.append(at)
            accs = [at[gofs : gofs + C, :] if gofs else at[:, :]
                    for at, gofs in zip(accs_t, (0, 32, 0, 32))]
            # schedule: group id per step
            gids = []
            pari = 0
            for i, (_, tt) in enumerate(splits):
                se_t = se_in_tile[i]
                for t in range(tt):
                    base = 0 if t < se_t else 2
                    gids.append(base + pari % 2)
                    pari += 1
            last_step = [-1, -1, -1, -1]
            for s, g in enumerate(gids):
                last_step[g] = s
            started = [False, False, False, False]
            step = 0
            bn_k = 0
            for i, (t0, tt) in enumerate(splits):
                eb = ebufs[i % NBUF]
                nc.sync.dma_start(
                    out=eb[:, :tt, :], in_=featv[:, t0 : t0 + tt, :]
                )
                oh = obufs[i % NOBUF]
                nc.vector.tensor_tensor(
                    out=oh[:, :, :tt],
                    in0=tgt_sb[:, t0 : t0 + tt].unsqueeze(1).to_broadcast(
                        [P, C, tt]
                    ),
                    in1=iota_cm[:, :, :tt],
                    op=mybir.AluOpType.is_equal,
                )
                se_t = se_in_tile[i]
                if se_t:
                    nc.scalar.activation(
                        out=se_scr[:, :se_t, :], in_=eb[:, :se_t, 0:D],
                        func=mybir.ActivationFunctionType.Square,
                        accum_out=sq_acc[:, i : i + 1],
                    )
                if se_t < tt:
                    ebf = eb[:, :, :].rearrange("p t e -> p (t e)")
                    lo = se_t * E
                    hi = tt * E
                    while lo < hi:
                        sz = min(512, hi - lo)
                        nc.vector.bn_stats(
                            out=bn_out[:, bn_k, :],
                            in_=ebf[:, lo : lo + sz],
                        )
                        bn_k += 1
                        lo += sz
                for t in range(tt):
                    g = gids[step]
                    nc.tensor.matmul(
                        accs[g],
                        lhsT=oh[:, :, t],
                        rhs=eb[:, t, :],
                        start=not started[g],
                        stop=(step == last_step[g]),
                        tile_position=(0, (g % 2) * 32),
                    )
                    started[g] = True
                    step += 1

            # ---- finale (tiny, per-core) ----
            # merge the four column groups; SE into cols 0:65, BN into
            # cols 65:130
            cgs = []
            for gi, gofs in enumerate((0, 32, 0, 32)):
                cg = fp.tile([gofs + C, E], f32, name=f"cg{gi}")
                nc.vector.tensor_copy(
                    cg[gofs : gofs + C, :] if gofs else cg[:, :],
                    accs[gi],
                )
                cgs.append(cg[gofs : gofs + C, :] if gofs else cg[:, :])
            accm = ps2.tile([C, 2 * E], f32)
            trps = ps2.tile([D, C], f32)
            misc = ps2.tile([C, 200], f32)
            nc.tensor.matmul(
                accm[:, 0:E], lhsT=eyes[0], rhs=cgs[0],
                start=True, stop=False,
            )
            nc.tensor.matmul(
                accm[:, 0:E], lhsT=eyes[1], rhs=cgs[1],
                start=False, stop=True,
            )
            nc.tensor.matmul(
                accm[:, E : 2 * E], lhsT=eyes[2], rhs=cgs[2],
                start=True, stop=False,
            )
            nc.tensor.matmul(
                accm[:, E : 2 * E], lhsT=eyes[3], rhs=cgs[3],
                start=False, stop=True,
            )
            # totals = SE + BN parts (stage PSUM halves through SBUF)
            accs_sb = fp.tile([C, 2 * E], f32)
            nc.vector.tensor_copy(accs_sb[:, :], accm[:, :])
            tot = fp.tile([C, E], f32)
            nc.vector.tensor_tensor(
                out=tot[:, :], in0=accs_sb[:, 0:E],
                in1=accs_sb[:, E : 2 * E], op=mybir.AluOpType.add,
            )
            fsum = tot[:, 0:D]  # [13, 64]
            ccnt = tot[:, D : D + 1]  # [13, 1]
            cbn = accs_sb[:, E + D : E + D + 1]  # BN per-class counts

            # per-class means and presence
            cmax = fp.tile([C, 1], f32)
            nc.vector.tensor_scalar(
                out=cmax[:, :], in0=ccnt, scalar1=1.0, scalar2=None,
                op0=mybir.AluOpType.max,
            )
            rec = fp.tile([C, 1], f32)
            nc.vector.reciprocal(rec[:, :], cmax[:, :])
            trin = fp.tile([C, D], f32)
            nc.vector.tensor_scalar(
                out=trin[:, :], in0=fsum, scalar1=rec[:, :], scalar2=None,
                op0=mybir.AluOpType.mult,
            )
            pres = fp.tile([C, 1], f32)
            nc.vector.tensor_scalar(
                out=pres[:, :], in0=ccnt, scalar1=0.0,
                scalar2=None, op0=mybir.AluOpType.is_gt,
            )

            # per-class dot(centers, fsum), ccnt*||c||^2 -> pack4 (+V_bn)
            junk0 = fp.tile([C, D], f32)
            cn2 = fp.tile([C, 1], f32)
            nc.vector.tensor_tensor(
                out=junk0[:, :], in0=cen_sb[:, :], in1=cen_sb[:, :],
                op=mybir.AluOpType.mult,
            )
            nc.vector.tensor_reduce(
                out=cn2[:, :], in_=junk0[:, :],
                axis=mybir.AxisListType.X, op=mybir.AluOpType.add,
            )
            pack4 = fp.tile([C, 4], f32)
            junk1 = fp.tile([C, D], f32)
            nc.vector.tensor_tensor(
                out=junk1[:, :], in0=cen_sb[:, :], in1=fsum,
                op=mybir.AluOpType.mult,
            )
            nc.vector.tensor_reduce(
                out=pack4[:, 0:1], in_=junk1[:, :],
                axis=mybir.AxisListType.X, op=mybir.AluOpType.add,
            )
            nc.vector.tensor_tensor(
                out=pack4[:, 1:2], in0=cn2[:, :], in1=ccnt,
                op=mybir.AluOpType.mult,
            )
            nc.vector.tensor_copy(pack4[:, 2:3], ccnt)
            nc.vector.tensor_copy(pack4[:, 3:4], cbn)

            # cross-partition sums: [Tdot, Utot, cnt_b, V_bn]
            red4 = misc[0:1, 4:8]
            nc.tensor.matmul(
                red4, lhsT=ones[0:C, :], rhs=pack4[:, :],
                start=True, stop=True,
            )

            # S_raw: ScalarE part from sq_acc; DVE part from bn_out:
            # count*var + count*mean^2 (even + odd)
            m2 = fp.tile([P, nbn, 2], f32)
            nc.vector.tensor_tensor(
                out=m2[:, :, 0:1], in0=bn_out[:, :, 1:2],
                in1=bn_out[:, :, 1:2], op=mybir.AluOpType.mult,
            )
            nc.vector.tensor_tensor(
                out=m2[:, :, 1:2], in0=bn_out[:, :, 4:5],
                in1=bn_out[:, :, 4:5], op=mybir.AluOpType.mult,
            )
            cm2 = fp.tile([P, nbn, 2], f32)
            nc.vector.tensor_tensor(
                out=cm2[:, :, 0:1], in0=m2[:, :, 0:1],
                in1=bn_out[:, :, 0:1], op=mybir.AluOpType.mult,
            )
            nc.vector.tensor_tensor(
                out=cm2[:, :, 1:2], in0=m2[:, :, 1:2],
                in1=bn_out[:, :, 3:4], op=mybir.AluOpType.mult,
            )
            sqv = fp.tile([P, nbn, 2], f32)
            nc.vector.tensor_tensor(
                out=sqv[:, :, 0:1], in0=cm2[:, :, 0:1],
                in1=bn_out[:, :, 2:3], op=mybir.AluOpType.add,
            )
            nc.vector.tensor_tensor(
                out=sqv[:, :, 1:2], in0=cm2[:, :, 1:2],
                in1=bn_out[:, :, 5:6], op=mybir.AluOpType.add,
            )
            red_bn = fp.tile([P, 1], f32)
            nc.vector.tensor_reduce(
                out=red_bn[:, :],
                in_=sqv[:, :, :].rearrange("p g s -> p (g s)"),
                axis=mybir.AxisListType.X, op=mybir.AluOpType.add,
            )
            red_sq = fp.tile([P, 1], f32)
            nc.vector.tensor_reduce(
                out=red_sq[:, :], in_=sq_acc[:, :],
                axis=mybir.AxisListType.X, op=mybir.AluOpType.add,
            )
            red_all = fp.tile([P, 1], f32)
            nc.vector.tensor_tensor(
                out=red_all[:, :], in0=red_sq[:, :], in1=red_bn[:, :],
                op=mybir.AluOpType.add,
            )
            s_ps = misc[0:1, 8:9]
            nc.tensor.matmul(
                s_ps, lhsT=ones[:, :], rhs=red_all[:, :],
                start=True, stop=True,
            )

            # transpose cmeans -> [64, 13]; present -> [1, 13]
            nc.tensor.transpose(trps[:, :], trin[:, :], ident[:, :])
            trsb = fp.tile([D, C], f32)
            nc.vector.tensor_copy(trsb[:, :], trps[:, :])
            cmT = trsb[0:D, :]  # [64, 13]
            prps = misc[0:1, 12 : 12 + C]
            nc.tensor.transpose(prps, pres[:, :], ident[:, :])
            presT = fp.tile([1, C], f32)
            nc.vector.tensor_copy(presT[:, :], prps)

            # pairwise squared distances between class means
            diff = fp.tile([D, C, C], f32)
            nc.vector.tensor_tensor(
                out=diff[:, :, :],
                in0=cmT.unsqueeze(2).to_broadcast([D, C, C]),
                in1=cmT.unsqueeze(1).to_broadcast([D, C, C]),
                op=mybir.AluOpType.subtract,
            )
            dsq = fp.tile([D, C, C], f32)
            nc.vector.tensor_tensor(
                out=dsq[:, :, :], in0=diff[:, :, :], in1=diff[:, :, :],
                op=mybir.AluOpType.mult,
            )
            dd2 = misc[0:1, 28 : 28 + C * C]
            nc.tensor.matmul(
                dd2, lhsT=ones[0:1, :],
                rhs=bigeye[:, :, :].rearrange("p a b -> p (a b)"),
                start=True, stop=False,
            )
            nc.tensor.matmul(
                dd2, lhsT=ones[0:D, :],
                rhs=dsq[:, :, :].rearrange("d a b -> d (a b)"),
                start=False, stop=True,
            )
            dist = fp.tile([1, C * C], f32)
            nc.scalar.activation(
                out=dist[:, :], in_=dd2,
                func=mybir.ActivationFunctionType.Sqrt,
            )
            hinge = fp.tile([1, C * C], f32)  # holds -relu(M - dist)
            nc.vector.tensor_scalar(
                out=hinge[:, :], in0=dist[:, :], scalar1=MARGIN,
                scalar2=MARGIN, op0=mybir.AluOpType.min,
                op1=mybir.AluOpType.subtract,
            )
            pm = fp.tile([1, C, C], f32)
            nc.vector.tensor_tensor(
                out=pm[:, :, :],
                in0=presT[:, :].unsqueeze(2).to_broadcast([1, C, C]),
                in1=presT[:, :].unsqueeze(1).to_broadcast([1, C, C]),
                op=mybir.AluOpType.mult,
            )
            pmf = pm[:, :, :].rearrange("p a b -> p (a b)")
            # raw per-batch sums; host does the final few divisions
            scal = fp.tile([1, 8], f32)
            nc.vector.memset(scal[:, 7:8], 0.0)
            terms = fp.tile([1, C * C], f32)
            nc.vector.tensor_tensor(
                out=terms[:, :], in0=hinge[:, :], in1=pmf,
                op=mybir.AluOpType.mult,
            )
            nc.vector.tensor_reduce(
                out=scal[:, 4:5], in_=terms[:, :],
                axis=mybir.AxisListType.X, op=mybir.AluOpType.add,
            )
            nc.vector.tensor_reduce(
                out=scal[:, 5:6], in_=presT[:, :], axis=mybir.AxisListType.X,
                op=mybir.AluOpType.add,
            )
            nc.vector.tensor_copy(scal[:, 0:1], s_ps)
            nc.vector.tensor_copy(scal[:, 1:4], red4[0:1, 0:3])
            nc.vector.tensor_copy(scal[:, 6:7], red4[0:1, 3:4])

            nc.sync.dma_start(out=out_h[:, :], in_=scal[:, :])
    nc.finalize()
    return nc


# set by test.py to capture profile info
TRACE = False
LAST = {}


def _ensure_ntff_hook():
    """The agent image's antenv lacks axon_hooks; synthesize it so
    run_bass_kernel_spmd(trace=True) can profile. Best-effort."""
    import sys
    import types

    try:
        from antenv.axon_hooks import get_axon_ntff_profile_hook  # noqa: F401
        return
    except ImportError:
        pass
    try:
        from trn_agent_boot.trn_boot import _ntff_profile_via_ctypes

        hook = _ntff_profile_via_ctypes("/opt/axon/libaxon_pjrt.so")
        mod = types.ModuleType("antenv.axon_hooks")
        mod._hook = hook
        mod.get_axon_ntff_profile_hook = lambda: mod._hook
        mod.set_axon_ntff_profile_hook = lambda h: setattr(mod, "_hook", h)
        sys.modules["antenv.axon_hooks"] = mod
        import antenv

        antenv.axon_hooks = mod
    except Exception as e:  # degrade: no profile, run still works
        print(f"ntff hook injection failed: {e}")


def kernel(pred=None, target=None, feat=None, batch=None, centers=None):
    target = np.asarray(target)
    feat = np.asarray(feat, dtype=np.float32)
    batch = np.asarray(batch)
    centers = np.asarray(centers, dtype=np.float32)
    N = feat.shape[0]

    # shard at batch boundaries: core b <- batch b (batch is sorted)
    bounds = np.searchsorted(batch, np.arange(B + 1))
    sizes = np.diff(bounds)
    T = int(max((int(sizes.max()) + P - 1) // P, TT))
    Npad = P * T

    in_maps = []
    for b in range(B):
        lo, hi = int(bounds[b]), int(bounds[b + 1])
        fb = np.zeros((Npad, E), dtype=ml_dtypes.bfloat16)
        tb = np.full((Npad,), C, dtype=np.int16)
        fb[: hi - lo, 0:D] = feat[lo:hi].astype(ml_dtypes.bfloat16)
        fb[: hi - lo, D] = 1.0
        tb[: hi - lo] = target[lo:hi]
        inv = tb == IGNORE
        if inv.any():
            tb[inv] = C  # one-hot miss -> excluded everywhere
            fb[inv] = 0.0  # excluded from S and the ones column
        in_maps.append({"feat": fb, "tgt": tb, "centers": centers})

    nc = build_nc(T)
    if TRACE:
        _ensure_ntff_hook()
    res = run_bass_kernel_spmd(nc, in_maps, list(range(N_CORES)), trace=TRACE)
    LAST["results"] = res

    rows = np.stack(
        [np.asarray(res.results[b]["out"]).reshape(8) for b in range(B)]
    ).astype(np.float64)
    s_raw, tdot, utot, cnt_b, tsum, kpres, v_bn = (
        rows[:, j] for j in range(7)
    )
    s = s_raw - v_bn  # bn_stats runs included the ones column
    npairs = kpres * (kpres - 1.0)
    intra = (s - 2.0 * tdot + utot) / np.maximum(cnt_b, 1.0)
    inter = -tsum / np.maximum(npairs, 1.0)
    present = cnt_b > 0
    den = max(float(present.sum()), 1.0)
    loss = LOSS_W * (
        INTRA_W * float(np.where(present, intra, 0.0).sum()) / den
        + INTER_W * float(np.where(present, inter, 0.0).sum()) / den
    )
    return np.float32(loss)
